# revision 55
# baseline (speedup 1.0000x reference)
"""GCN critic (2x GCNConv + 2 MLP heads) on 8 trn2 NeuronCores.

Sharding: destination-node blocks of 1250 nodes per core. Edges bucketed by
dst window (128 dst nodes). Per window, sources are DEDUPLICATED on the host
and the segment-sum matrix S (multi-hot: S[u,d] = #edges(src=u, dst=d), fp8
e4m3 -- small ints are exact) is built host-side and streamed in; the kernel
gathers each unique source row once (256B rows) and accumulates msg.T @ S
per 128-row chunk on the tensor engine into PSUM (feature-major segments).

conv1 gathers raw (dis-scaled) input features from a replicated table; the
w1 matmul is applied after the segment-sum (linearity). conv2 gathers
dis*relu(out1) rows from an AllGather'ed table; the same unique sources / S
/ index table serve both convs.

Node tables use an exchange-friendly layout: windows are AllGather'ed in
groups of CC_GROUP (rank-major within a group), overlapped with conv1.
Within each window the unique sources are split FRONT (table rows < HSPLIT,
covered by the first two AllGathers) and BACK (the rest), and conv2 runs in
two phases: front partial segment-sums (start as soon as AG1 lands, while
later AGs are still in flight) are stashed in SBUF, then back chunks are
accumulated and combined. Each gather's in_ap is restricted to the rows its
phase needs so the tile framework only waits on the relevant AllGathers
(back-slot indices are stored relative to HSPLIT).

The transpose/store chain runs one window behind the compute to keep the PE
stream bubble-free; x2d stores ride the scalar engine's DMA path so the
sync queue (constants) never blocks them. q outputs are written as
[128, NWIN] in one DMA each and reshaped on the host.
"""

import numpy as np
import ml_dtypes

BF16 = ml_dtypes.bfloat16
FP8 = ml_dtypes.float8_e4m3fn
N_NODES = 10000
OBS_DIM = 30
ACT_DIM = 4
HID = 128
N_CORES = 8
BLK = N_NODES // N_CORES  # 1250 dst nodes per core
P = 128
NWIN = (BLK + P - 1) // P  # 10 windows per core (last is 98 wide)
GMAX = 1024  # max idx per dma_gather instruction (HW ucode limit)
XCOLS = 128  # conv1 gather row (bf16): 34 used, pad to 256B
NROWS = NWIN * N_CORES * P  # 10240 table rows
CC_GROUP = 2  # windows per AllGather (5 AGs); groups 0-2 = front rows
FRONT_W = 6  # front = windows 0-5 (first 3 AllGather groups)
HSPLIT = FRONT_W * N_CORES * P  # front/back row split


def _remap(n):
    """node id -> AllGather'ed table row (rank-major within each group)."""
    c, r = n // BLK, n % BLK
    w, p = r // P, r % P
    g, wi = w // CC_GROUP, w % CC_GROUP
    return g * CC_GROUP * N_CORES * P + c * CC_GROUP * P + wi * P + p


def _prep_graph(edge_index):
    """Host-side index preprocessing (the sharding step).

    Slots per (core, window): [front chunks | back chunks], chunk counts
    padded to the max over cores (SPMD). Returns wrapped idx (back slots
    relative to HSPLIT), fp8 multi-hot S, per-window front/back chunk
    counts, and deg^-1/2.
    """
    src = np.asarray(edge_index[0], dtype=np.int64)
    dst = np.asarray(edge_index[1], dtype=np.int64)
    loops = np.arange(N_NODES, dtype=np.int64)
    src = np.concatenate([src, loops])
    dst = np.concatenate([dst, loops])
    deg = np.bincount(dst, minlength=N_NODES).astype(np.float32)
    dis = (1.0 / np.sqrt(np.maximum(deg, 1.0))).astype(np.float32)

    srcm = _remap(src)
    uniq = {}
    fmax = np.zeros(NWIN, dtype=np.int64)
    bmax = np.zeros(NWIN, dtype=np.int64)
    for c in range(N_CORES):
        for w in range(NWIN):
            lo = c * BLK + w * P
            wlen = min(P, BLK - w * P)
            m = (dst >= lo) & (dst < lo + wlen)
            u, inv = np.unique(srcm[m], return_inverse=True)
            nf = int(np.searchsorted(u, HSPLIT))
            uniq[c, w] = (u, inv, (dst[m] - lo).astype(np.int64), nf)
            fmax[w] = max(fmax[w], nf)
            bmax[w] = max(bmax[w], len(u) - nf)
    fch = (fmax + P - 1) // P  # front chunks per window
    bch = (bmax + P - 1) // P  # back chunks per window

    tot_chunks = int((fch + bch).sum())
    tot_e = tot_chunks * P
    idx_all = np.zeros((N_CORES, tot_e), np.int64)
    S_all = np.zeros((N_CORES, tot_chunks, P, P), np.float32)  # [chunk, u, d]
    for c in range(N_CORES):
        off = 0
        coff = 0
        for w in range(NWIN):
            u, inv, dloc, nf = uniq[c, w]
            # slot of unique-src j: front j<nf at j, back at fch*P + (j-nf)
            slot = np.where(np.arange(len(u)) < nf,
                            np.arange(len(u)),
                            fch[w] * P + np.arange(len(u)) - nf)
            idx_all[c, off + slot] = u - np.where(np.arange(len(u)) < nf,
                                                  0, HSPLIT)
            es = slot[inv]  # edge -> slot
            np.add.at(S_all[c], (coff + es // P, es % P, dloc), 1.0)
            off += (fch[w] + bch[w]) * P
            coff += fch[w] + bch[w]
    # wrap idx: position i -> partition i%16, col i//16; replicate to 8 groups
    pos = np.arange(tot_e)
    idx_wrap = np.zeros((N_CORES, P, tot_e // 16), np.int16)
    for g in range(8):
        idx_wrap[:, g * 16 + pos % 16, pos // 16] = idx_all.astype(np.int16)
    S_in = S_all.transpose(0, 2, 1, 3).reshape(N_CORES, P, tot_chunks * P)
    return idx_wrap, S_in.astype(FP8), fch, bch, dis


def _build(fch, bch):
    import concourse.bacc as bacc
    import concourse.mybir as mybir
    from concourse.tile import TileContext
    from concourse import library_config

    dt = mybir.dt
    tot_chunks = int((fch + bch).sum())
    tot_e = tot_chunks * P

    nc = bacc.Bacc(None, target_bir_lowering=False, num_devices=N_CORES,
                   num_swdge_queues=4)
    # ---- inputs ----
    x_dis = nc.dram_tensor("x_dis", [NROWS, XCOLS], dt.bfloat16, kind="ExternalInput")
    idx_in = nc.dram_tensor("idx", [P, tot_e // 16], dt.int16, kind="ExternalInput")
    S_dram = nc.dram_tensor("Sp", [P, tot_chunks * P], dt.float8e4, kind="ExternalInput")
    disb_in = nc.dram_tensor("disb", [P, NWIN * P], dt.float32, kind="ExternalInput")
    w1_in = nc.dram_tensor("w1p", [XCOLS, HID], dt.float32, kind="ExternalInput")
    w2_in = nc.dram_tensor("w2", [HID, HID], dt.float32, kind="ExternalInput")
    b1_in = nc.dram_tensor("b1c", [P, 1], dt.float32, kind="ExternalInput")
    b2_in = nc.dram_tensor("b2c", [P, 1], dt.float32, kind="ExternalInput")
    wq1a_in = nc.dram_tensor("wq1a", [HID, HID], dt.float32, kind="ExternalInput")
    wq2a_in = nc.dram_tensor("wq2a", [HID, HID], dt.float32, kind="ExternalInput")
    a1b_in = nc.dram_tensor("a1b", [P, HID], dt.float32, kind="ExternalInput")
    a2b_in = nc.dram_tensor("a2b", [P, HID], dt.float32, kind="ExternalInput")
    w1bb_in = nc.dram_tensor("w1bb", [P, HID], dt.float32, kind="ExternalInput")
    w2bb_in = nc.dram_tensor("w2bb", [P, HID], dt.float32, kind="ExternalInput")
    bq_in = nc.dram_tensor("bq", [P, 2], dt.float32, kind="ExternalInput")
    ident_in = nc.dram_tensor("ident", [P, P], dt.float32, kind="ExternalInput")
    q1_out = nc.dram_tensor("q1", [P, NWIN], dt.float32, kind="ExternalOutput")
    q2_out = nc.dram_tensor("q2", [P, NWIN], dt.float32, kind="ExternalOutput")

    with TileContext(nc) as tc:
        with tc.tile_pool(name="const", bufs=1) as cp, \
             tc.tile_pool(name="msgp", bufs=6) as msgp, \
             tc.tile_pool(name="work", bufs=3) as wp, \
             tc.tile_pool(name="xstage", bufs=4) as xsp, \
             tc.tile_pool(name="psum", bufs=3, space="PSUM") as pp, \
             tc.tile_pool(name="psum2", bufs=3, space="PSUM") as pp2, \
             tc.tile_pool(name="psum3", bufs=2, space="PSUM") as pp3, \
             tc.tile_pool(name="dram", bufs=1, space="DRAM") as dramp:

            x2d_local = dramp.tile([NWIN * P, HID], dt.bfloat16)
            x2d_full = dramp.tile([NROWS, HID], dt.bfloat16)

            # tiny warmup collective FIRST on gpsimd: starts the CC bootstrap
            # barrier (absorbs inter-core launch skew) as early as possible
            cc_wu_in = dramp.tile([P, 16], dt.bfloat16)
            cc_wu_out = dramp.tile([N_CORES * P, 16], dt.bfloat16)
            wu_sb = xsp.tile([P, 16], dt.bfloat16, tag="wu")
            nc.vector.memset(wu_sb[:], 0.0)
            nc.scalar.dma_start(cc_wu_in[:], wu_sb[:])
            nc.gpsimd.collective_compute(
                "AllGather", mybir.AluOpType.bypass,
                replica_groups=[list(range(N_CORES))],
                ins=[cc_wu_in[:].opt()], outs=[cc_wu_out[:].opt()])
            nc.gpsimd.load_library(library_config.mlp)

            # ---- load constants (gather/matmul deps first) ----
            idx_t = cp.tile([P, tot_e // 16], dt.int16)
            nc.sync.dma_start(idx_t[:], idx_in[:])
            # S streamed per window so early windows unblock fast
            S_t = cp.tile([P, tot_chunks, P], dt.float8e4)
            c0 = 0
            for w in range(NWIN):
                nch = int(fch[w] + bch[w])
                nc.sync.dma_start(
                    S_t[:, c0:c0 + nch, :],
                    S_dram[:, c0 * P:(c0 + nch) * P].rearrange(
                        "p (k d) -> p k d", d=P))
                c0 += nch
            w1_t = cp.tile([XCOLS, HID], dt.float32)
            nc.sync.dma_start(w1_t[:], w1_in[:])
            disb_t = cp.tile([P, NWIN * P], dt.float32)
            nc.sync.dma_start(disb_t[:], disb_in[:])
            b1_t = cp.tile([P, 1], dt.float32)
            nc.sync.dma_start(b1_t[:], b1_in[:])
            ident_t = cp.tile([P, P], dt.float32)
            nc.sync.dma_start(ident_t[:], ident_in[:])
            w2_t = cp.tile([HID, HID], dt.float32)
            nc.sync.dma_start(w2_t[:], w2_in[:])
            b2_t = cp.tile([P, 1], dt.float32)
            nc.sync.dma_start(b2_t[:], b2_in[:])
            # head consts only needed in conv2 -- loaded last
            wq1a_t = cp.tile([HID, HID], dt.float32)
            nc.sync.dma_start(wq1a_t[:], wq1a_in[:])
            wq2a_t = cp.tile([HID, HID], dt.float32)
            nc.sync.dma_start(wq2a_t[:], wq2a_in[:])
            a1b_t = cp.tile([P, HID], dt.float32)
            nc.sync.dma_start(a1b_t[:], a1b_in[:])
            a2b_t = cp.tile([P, HID], dt.float32)
            nc.sync.dma_start(a2b_t[:], a2b_in[:])
            w1bb_t = cp.tile([P, HID], dt.float32)
            nc.sync.dma_start(w1bb_t[:], w1bb_in[:])
            w2bb_t = cp.tile([P, HID], dt.float32)
            nc.sync.dma_start(w2bb_t[:], w2bb_in[:])
            bq_t = cp.tile([P, 2], dt.float32)
            nc.sync.dma_start(bq_t[:], bq_in[:])

            q1_col = cp.tile([P, NWIN], dt.float32)
            q2_col = cp.tile([P, NWIN], dt.float32)
            segf_sb = cp.tile([HID, NWIN, P], dt.float32)  # conv2 front partials

            qn = [0]

            def gather_run(table, lo, hi, c0, nchunks, ecols):
                """Gather nchunks*P slots; table rows restricted to [lo,hi)."""
                msg = msgp.tile([P, nchunks, ecols], dt.bfloat16, tag="msg")
                e0 = c0 * P
                n_left = nchunks * P
                off = 0
                while n_left > 0:
                    g = min(n_left, GMAX)
                    nc.gpsimd.dma_gather(
                        out_ap=msg[:, off // P:(off + g) // P, :],
                        in_ap=table[lo:hi, :],
                        idxs_ap=idx_t[:, (e0 + off) // 16:(e0 + off + g) // 16],
                        num_idxs=g, num_idxs_reg=g, elem_size=ecols,
                        queue_num=qn[0] % 4,
                    )
                    qn[0] += 1
                    off += g
                    n_left -= g
                return msg

            def scatter_run(seg, msg, c0, nchunks, start, stop):
                """Accumulate multi-hot matmuls into seg psum."""
                for k in range(nchunks):
                    nc.tensor.matmul(out=seg[:], lhsT=msg[:, k, :],
                                     rhs=S_t[:, c0 + k, :],
                                     start=(start and k == 0),
                                     stop=(stop and k == nchunks - 1))

            def issue_cc(g):
                w0 = g * CC_GROUP
                w1 = min(w0 + CC_GROUP, NWIN)
                nc.gpsimd.collective_compute(
                    "AllGather", mybir.AluOpType.bypass,
                    replica_groups=[list(range(N_CORES))],
                    ins=[x2d_local[w0 * P:w1 * P, :].opt()],
                    outs=[x2d_full[w0 * N_CORES * P:w1 * N_CORES * P, :].opt()])

            def flush_x2d(w, x2d):
                """Transpose x2d (f-major) to node-major, store, AllGather."""
                x2d_tp = pp3.tile([P, HID], dt.float32, space="PSUM", tag="tp")
                nc.tensor.transpose(out=x2d_tp[:], in_=x2d[:], identity=ident_t[:])
                x2d_sb = xsp.tile([P, HID], dt.bfloat16, tag="x2s")
                nc.scalar.copy(x2d_sb[:], x2d_tp[:])
                nc.scalar.dma_start(x2d_local[w * P:(w + 1) * P, :], x2d_sb[:])
                # trigger group g two windows after its last store so the
                # gpsimd-stream trigger never stalls the gather pipeline
                if w >= 3 and (w - 3) % CC_GROUP == 0:
                    issue_cc((w - 3) // CC_GROUP)

            # ========== conv1 (transpose chain one window behind) ==========
            c0 = 0
            pend = None  # (w, x2d tile) not yet flushed
            for w in range(NWIN):
                nf, nb = int(fch[w]), int(bch[w])
                msgf = gather_run(x_dis, 0, HSPLIT, c0, nf, XCOLS)
                msgb = gather_run(x_dis, HSPLIT, NROWS, c0 + nf, nb, XCOLS)
                segx = pp.tile([XCOLS, P], dt.float32, space="PSUM", tag="seg")
                scatter_run(segx, msgf, c0, nf, True, False)
                scatter_run(segx, msgb, c0 + nf, nb, False, True)
                segx_sb = wp.tile([XCOLS, P], dt.float32, tag="segx")
                nc.scalar.copy(segx_sb[:], segx[:])
                o1 = pp2.tile([HID, P], dt.float32, space="PSUM", tag="mm")
                nc.tensor.matmul(out=o1[:], lhsT=w1_t[:], rhs=segx_sb[:],
                                 start=True, stop=True)  # [128f, 128d] fm
                if pend is not None:
                    flush_x2d(*pend)
                t1 = wp.tile([HID, P], dt.float32, tag="t1")
                nc.vector.tensor_mul(t1[:], o1[:], disb_t[:, w * P:w * P + P])
                x2 = wp.tile([HID, P], dt.float32, tag="x2")
                nc.scalar.activation(x2[:], t1[:], mybir.ActivationFunctionType.Relu,
                                     bias=b1_t[:], scale=1.0)
                x2d = wp.tile([HID, P], dt.float32, tag="x2d")
                nc.vector.tensor_mul(x2d[:], x2[:], disb_t[:, w * P:w * P + P])
                pend = (w, x2d)
                c0 += nf + nb
            flush_x2d(*pend)
            issue_cc(NWIN // CC_GROUP - 1)

            # ========== conv2 phase F: front partial segment sums ==========
            c0 = 0
            for w in range(NWIN):
                nf, nb = int(fch[w]), int(bch[w])
                msgf = gather_run(x2d_full, 0, HSPLIT, c0, nf, HID)
                segf = pp.tile([HID, P], dt.float32, space="PSUM", tag="seg")
                scatter_run(segf, msgf, c0, nf, True, True)
                nc.scalar.copy(segf_sb[:, w, :], segf[:])
                c0 += nf + nb

            # ========== conv2 phase B: back chunks + combine + heads ========
            c0 = 0
            for w in range(NWIN):
                nf, nb = int(fch[w]), int(bch[w])
                msgb = gather_run(x2d_full, HSPLIT, NROWS, c0 + nf, nb, HID)
                segb = pp.tile([HID, P], dt.float32, space="PSUM", tag="seg")
                scatter_run(segb, msgb, c0 + nf, nb, True, True)
                seg2_sb = wp.tile([HID, P], dt.float32, tag="seg2")
                nc.vector.tensor_add(seg2_sb[:], segb[:], segf_sb[:, w, :])
                o2 = pp2.tile([HID, P], dt.float32, space="PSUM", tag="mm")
                nc.tensor.matmul(out=o2[:], lhsT=w2_t[:], rhs=seg2_sb[:],
                                 start=True, stop=True)
                t2 = wp.tile([HID, P], dt.float32, tag="t2")
                nc.vector.tensor_mul(t2[:], o2[:], disb_t[:, w * P:w * P + P])
                x3 = wp.tile([HID, P], dt.float32, tag="x3")
                nc.scalar.activation(x3[:], t2[:], mybir.ActivationFunctionType.Relu,
                                     bias=b2_t[:], scale=1.0)
                # heads: h = relu(x3.T @ wqa + a); q = sum(h * wbb) + bq
                for (wqa_t, ab_t, wbb_t, qcol, bqi) in (
                        (wq1a_t, a1b_t, w1bb_t, q1_col, 0),
                        (wq2a_t, a2b_t, w2bb_t, q2_col, 1)):
                    hp = pp2.tile([P, HID], dt.float32, space="PSUM", tag="mm")
                    nc.tensor.matmul(out=hp[:], lhsT=x3[:], rhs=wqa_t[:],
                                     start=True, stop=True)  # [d, f']
                    hb = wp.tile([P, HID], dt.float32, tag="hb")
                    nc.vector.tensor_add(hb[:], hp[:], ab_t[:])
                    hr = wp.tile([P, HID], dt.float32, tag="hr")
                    nc.scalar.activation(hr[:], hb[:], mybir.ActivationFunctionType.Relu)
                    hw = wp.tile([P, HID], dt.float32, tag="hw")
                    nc.vector.tensor_mul(hw[:], hr[:], wbb_t[:])
                    nc.vector.tensor_reduce(
                        out=qcol[:, w:w + 1], in_=hw[:], op=mybir.AluOpType.add,
                        axis=mybir.AxisListType.X)
                c0 += nf + nb

            qb1 = wp.tile([P, NWIN], dt.float32, tag="qb1")
            nc.vector.tensor_scalar(out=qb1[:], in0=q1_col[:], scalar1=bq_t[:, 0:1],
                                    scalar2=None, op0=mybir.AluOpType.add)
            qb2 = wp.tile([P, NWIN], dt.float32, tag="qb2")
            nc.vector.tensor_scalar(out=qb2[:], in0=q2_col[:], scalar1=bq_t[:, 1:2],
                                    scalar2=None, op0=mybir.AluOpType.add)
            nc.scalar.dma_start(q1_out[:], qb1[:])
            nc.scalar.dma_start(q2_out[:], qb2[:])

    nc.compile()
    return nc


_CACHE = {}


def kernel(obs, action, edge_index,
           w_g1, b_g1, w_g2, b_g2,
           w_q1a, b_q1a, w_q1b, b_q1b,
           w_q2a, b_q2a, w_q2b, b_q2b, _trace=False):
    from concourse.bass_utils import run_bass_kernel_spmd

    obs = np.asarray(obs, np.float32)
    action = np.asarray(action, np.float32)
    idx_wrap, S_in, fch, bch, dis = _prep_graph(np.asarray(edge_index))

    key = (tuple(fch.tolist()), tuple(bch.tolist()))
    if key not in _CACHE:
        _CACHE[key] = _build(fch, bch)
    nc = _CACHE[key]

    x = np.concatenate([obs, action], axis=1) * dis[:, None]
    x_dis = np.zeros((NROWS, XCOLS), BF16)
    rows = _remap(np.arange(N_NODES))
    x_dis[rows, :OBS_DIM + ACT_DIM] = x.astype(BF16)
    w1p = np.zeros((XCOLS, HID), np.float32)
    w1p[:OBS_DIM + ACT_DIM, :] = np.asarray(w_g1, np.float32)
    ident = np.eye(P, dtype=np.float32)
    bq = np.zeros((P, 2), np.float32)
    bq[:, 0] = float(np.asarray(b_q1b).reshape(-1)[0])
    bq[:, 1] = float(np.asarray(b_q2b).reshape(-1)[0])

    in_maps = []
    for c in range(N_CORES):
        disp = np.zeros(NWIN * P, np.float32)
        disp[:BLK] = dis[c * BLK:(c + 1) * BLK]
        disb = np.broadcast_to(disp[None, :], (P, NWIN * P)).copy()
        in_maps.append(dict(
            x_dis=x_dis, idx=idx_wrap[c], Sp=S_in[c],
            disb=disb, w1p=w1p, w2=np.asarray(w_g2, np.float32),
            b1c=np.asarray(b_g1, np.float32).reshape(P, 1),
            b2c=np.asarray(b_g2, np.float32).reshape(P, 1),
            wq1a=np.asarray(w_q1a, np.float32), wq2a=np.asarray(w_q2a, np.float32),
            a1b=np.broadcast_to(np.asarray(b_q1a, np.float32)[None, :], (P, HID)).copy(),
            a2b=np.broadcast_to(np.asarray(b_q2a, np.float32)[None, :], (P, HID)).copy(),
            w1bb=np.broadcast_to(np.asarray(w_q1b, np.float32).reshape(-1)[None, :], (P, HID)).copy(),
            w2bb=np.broadcast_to(np.asarray(w_q2b, np.float32).reshape(-1)[None, :], (P, HID)).copy(),
            bq=bq, ident=ident,
        ))
    res = run_bass_kernel_spmd(nc, in_maps, core_ids=list(range(N_CORES)),
                               trace=_trace)
    q1 = np.concatenate([res.results[c]["q1"].T.reshape(-1)[:BLK]
                         for c in range(N_CORES)], axis=0)[:, None]
    q2 = np.concatenate([res.results[c]["q2"].T.reshape(-1)[:BLK]
                         for c in range(N_CORES)], axis=0)[:, None]
    kernel._last_exec_ns = res.exec_time_ns
    return (q1, q2)
